# revision 71
# baseline (speedup 1.0000x reference)
"""Trainium2 Bass kernel for nn_ABCFramework_17755394802208.

Conv backbone (6x 3x3 SAME convs + 2 maxpools, 256^2 -> 64^2) feeding full
self-attention over N=4096 tokens with a Swin-style relative-position bias.

Sharding: 8 cores = (batch b in {0,1}) x (head h in {0..3}); each core runs the
conv backbone for its batch, projects q/k/v for its head, and computes full
attention for its (b, h). Output slices are gathered/reassembled on host.

Conv formulation: each matmul processes G image rows at once (G divides H).
Odd layers (1,3,5) load a moving tile [(ci, g') x (strip, padded col)] from a
DRAM plane -- one DMA per input channel -- and accumulate 9 (or 3 for Cin=1)
zero-padded block-diagonal stationaries over the taps in PSUM. Even layers
(2,4,6) share G with their predecessor and read the predecessor's SBUF output
tile [(g, co) x (strip, padded col)] DIRECTLY as the moving operand: the row
shift of each tap lives in the stationary's diagonal offset (15 stationaries:
9 in-strip taps + 6 strip-boundary taps accumulated into PSUM subranges), so
conv pairs need no DRAM round-trip at all. Only the pool inputs/outputs touch
DRAM planes (channel-interleaved layout [row, C, Wp] -> one write per layer).
All matmuls run in bf16.

Attention: logits S^T = K_c^T Q_j into a 3-bank PSUM tile (3 key-chunks), one
wide exp on the Scalar engine per 3 chunks, then the relative-position bias is
applied multiplicatively on the Vector engine: exp(s+b) = exp(s) * EB where
EB = exp(bias atlas) is computed once on-chip. AV accumulates in PSUM with an
extra ones-row in V giving the softmax denominators; the final divide uses a
gpsimd partition_broadcast of the reciprocal row. Q and K project in a single
fused matmul per chunk (the 1/sqrt(d) scale is folded into the host weights).
"""
import sys

sys.path.insert(0, '/opt/trn_rl_repo')

import numpy as np

try:
    from ml_dtypes import bfloat16 as BF16_NP
except ImportError:  # pragma: no cover
    import jax.numpy as _jnp
    BF16_NP = _jnp.bfloat16

NUM_HEADS = 4
DIM_HEAD = 64
TABLE_M = 160
B = 2
N = 4096          # tokens (64 x 64)
GRID = 64
NCH = 32          # m-chunks of 128 keys
NQC = 8           # n-chunks of 512 queries
CF = 8192         # 127 * 64 compact-table window length

# conv layer configs: (Cin, Cout, H, W, G rows-per-group); G divides H and is
# shared within each (odd, even) pair so even layers fuse onto the odd output
CONVS = [(1, 3, 256, 256, 32), (3, 3, 256, 256, 32),
         (3, 6, 128, 128, 16), (6, 6, 128, 128, 16),
         (6, 9, 64, 64, 8), (9, 9, 64, 64, 8)]

# plane name -> (C, H, W); per-channel sections, each [row, Wp] with guards
PLANES = {
    'x':  (1, 256, 256),
    'm2': (3, 256, 256),
    'p1': (3, 128, 128),
    'm4': (6, 128, 128),
    'p2': (6, 64, 64),
}


def _ntap(li):
    if li == 0:
        return 3
    return 15 if li % 2 else 9


def _wshape(li):
    Cin, Cout, _, _, G = CONVS[li]
    if li == 0:
        return 3 * G, Cout * G
    if li % 2:  # even layer (fused): rows = predecessor's output partitions
        return G * Cin, Cout * G
    return Cin * (G + 2), Cout * G


def _lay(C, H, W):
    # per-channel section geometry: Wp, guard offset, section len, total len
    Wp = W + 2
    Goff = Wp + 1
    blc = (H + 2) * Wp + 2 * Goff
    return Wp, Goff, blc, C * blc


_BUILD_CACHE = {}


def _build():
    if 'nc' in _BUILD_CACHE:
        return _BUILD_CACHE['nc']
    import concourse.bass as bass
    import concourse.mybir as mybir
    import concourse.tile as tile
    from concourse import bacc

    F32 = mybir.dt.float32
    BF16 = mybir.dt.bfloat16
    AF = mybir.ActivationFunctionType
    ALU = mybir.AluOpType

    nc = bacc.Bacc("TRN2", target_bir_lowering=False, debug=False, num_devices=8)

    # ---- external inputs (per-core shards prepared on host) ----
    _, _, _, BLX = _lay(*PLANES['x'])
    x_d = nc.dram_tensor("x", [1, BLX], BF16, kind="ExternalInput")
    wk_d, bx_d = [], []
    for i in range(6):
        rows, cols = _wshape(i)
        wk_d.append(nc.dram_tensor(f"w{i}", [rows, _ntap(i) * cols], BF16,
                                   kind="ExternalInput"))
        bx_d.append(nc.dram_tensor(f"bx{i}", [cols], F32, kind="ExternalInput"))
    wqkv_d = nc.dram_tensor("wqkv", [9, 320], BF16, kind="ExternalInput")
    wa_d = nc.dram_tensor("watlas", [128, CF], BF16, kind="ExternalInput")
    out_d = nc.dram_tensor("out", [64, N], F32, kind="ExternalOutput")

    with tile.TileContext(nc) as tc:
        with tc.tile_pool(name="const", bufs=1) as const, \
             tc.tile_pool(name="work", bufs=2) as work, \
             tc.tile_pool(name="dram", bufs=1, space="DRAM") as dram:

            # ---------------- constant tiles ----------------
            wkt, bxt = [], []
            for i in range(6):
                rows, cols = _wshape(i)
                wkt.append(const.tile([rows, _ntap(i), cols], BF16,
                                      tag=f"w{i}", name=f"w{i}"))
                bxt.append(const.tile([cols, 1], F32, tag=f"bx{i}",
                                      name=f"bx{i}"))

            zeros_bf = const.tile([128, 512], BF16, tag="zbf")
            nc.vector.memset(zeros_bf, 0.0)

            # prefetch all constants on the scalar queue (layer order) so the
            # sync queue only carries the latency-critical mov loads
            for i in range(6):
                nc.scalar.dma_start(out=wkt[i], in_=wk_d[i][:, :])
                nc.scalar.dma_start(out=bxt[i], in_=bx_d[i][:, None])
            wqkv_t = const.tile([9, 320], BF16, tag="wqkv")
            nc.scalar.dma_start(out=wqkv_t, in_=wqkv_d[:, :])

            # ---------------- DRAM planes + zero scratch ----------------
            geom, plane = {}, {}
            for nm, (C, H, Wd) in PLANES.items():
                geom[nm] = _lay(C, H, Wd)
                if nm != 'x':
                    plane[nm] = dram.tile([1, geom[nm][3]], BF16, tag=nm,
                                          name=nm)
            # token tile [9, 64(y), 64(x)], filled by conv6 via SBUF->SBUF DMA
            tokT = const.tile([9, GRID, GRID], BF16, tag="tok")
            zs = dram.tile([1, 8192], BF16, tag="zs", name="zs")
            nc.gpsimd.dma_start(
                out=bass.AP(tensor=zs.tensor, offset=0, ap=[[64, 128], [1, 64]]),
                in_=zeros_bf[0:128, 0:64])

            def zfill(tensor, offset, ap):
                total = 1
                for _, cnt in ap:
                    total *= cnt
                assert total <= 8192, total
                nc.gpsimd.dma_start(
                    out=bass.AP(tensor=tensor, offset=offset, ap=ap),
                    in_=bass.AP(tensor=zs.tensor, offset=0, ap=[[1, total]]))

            # guards: top row + bottom guard row for every channel section;
            # pool-output planes also need column guards
            for nm in ['m2', 'p1', 'm4', 'p2']:
                C, H, Wd = PLANES[nm]
                Wp, Goff, blc, bl = geom[nm]
                t = plane[nm].tensor
                zfill(t, 0, [[blc, C], [1, Goff + Wp]])
                zfill(t, Goff + (H + 1) * Wp, [[blc, C], [1, Wp + Goff]])
                if nm in ('p1', 'p2'):
                    zfill(t, Goff + Wp, [[blc, C], [Wp, H]])
                    zfill(t, Goff + 2 * Wp - 1, [[blc, C], [Wp, H]])

            # ---------------- conv backbone ----------------
            def conv_pair(lo_li, in_nm, out_nm):
                """Odd layer lo_li from plane in_nm, fused even layer lo_li+1
                writing plane out_nm (or 'tok')."""
                Cin, Cmid, H, Wd, G = CONVS[lo_li]
                Cout = CONVS[lo_li + 1][1]
                Wp, Goff, blc, bl = geom[in_nm]
                Sf = H // G
                mode3 = (lo_li == 0)
                rows = 3 * G if mode3 else Cin * (G + 2)
                colsA = Cmid * G          # odd output partitions (g, co)
                colsB = Cout * G
                in_t = x_d if in_nm == 'x' else plane[in_nm].tensor
                Wdo = Wd + 2

                # ---- odd layer: DMA moving tile, 3/9-tap matmuls ----
                mov = work.tile([rows, Sf, Wp], BF16, tag="mov", bufs=2,
                                name=f"mov{lo_li}")
                if mode3:
                    for ky in range(3):
                        src = bass.AP(tensor=in_t, offset=Goff + ky * Wp,
                                      ap=[[Wp, G], [G * Wp, Sf], [1, Wp]])
                        nc.sync.dma_start(
                            out=mov[ky * G:(ky + 1) * G, :, :], in_=src)
                else:
                    # halve only the bigger sections; tiny transfers are
                    # trigger-bound and one DMA per channel wins
                    halves = ((0, Sf // 2, nc.sync),
                              (Sf // 2, Sf - Sf // 2, nc.scalar)) \
                        if Wp > 100 else ((0, Sf, None),)
                    for ci in range(Cin):
                        pb = ci * (G + 2)
                        for s0, ns, eng in halves:
                            if eng is None:
                                eng = nc.sync if ci % 2 else nc.scalar
                            src = bass.AP(
                                tensor=in_t,
                                offset=ci * blc + Goff + s0 * G * Wp,
                                ap=[[Wp, G + 2], [G * Wp, ns], [1, Wp]])
                            eng.dma_start(
                                out=mov[pb:pb + G + 2, s0:s0 + ns, :], in_=src)

                outA = work.tile([colsA, Sf, Wdo], BF16, tag="out", bufs=2,
                                 name=f"outA{lo_li}")
                nc.vector.memset(outA[:, :, 0:1], 0.0)
                nc.vector.memset(outA[:, :, Wdo - 1:Wdo], 0.0)

                ns_max = 512 // Wd
                chs = [(s, min(ns_max, Sf - s)) for s in range(0, Sf, ns_max)]
                ntapA = 3 if mode3 else 9
                for g0 in range(0, len(chs), 3):
                    grp = chs[g0:g0 + 3]
                    pts = [psc.tile([colsA, 512], F32, tag="cps",
                                    name=f"cpsA{lo_li}_{g0}_{gi}")
                           for gi in range(len(grp))]
                    for t in range(ntapA):
                        kx = t if mode3 else t % 3
                        for pt, (s0, ns) in zip(pts, grp):
                            nc.tensor.matmul(
                                pt[:, 0:ns * Wd], wkt[lo_li][:, t, :],
                                mov[:, s0:s0 + ns, kx:kx + Wd],
                                start=(t == 0), stop=(t == ntapA - 1))
                    for pt, (s0, ns) in zip(pts, grp):
                        nc.vector.scalar_tensor_tensor(
                            out=outA[:, s0:s0 + ns, 1:1 + Wd],
                            in0=pt[:, 0:ns * Wd],
                            scalar=bxt[lo_li], in1=zeros_bf[0:colsA, 0:ns * Wd],
                            op0=ALU.add, op1=ALU.max)

                # ---- even layer: moving operand = outA (SBUF), 15 taps ----
                li = lo_li + 1
                outB = work.tile([colsB, Sf, Wdo], BF16, tag="out", bufs=2,
                                 name=f"outB{li}")
                nc.vector.memset(outB[:, :, 0:1], 0.0)
                nc.vector.memset(outB[:, :, Wdo - 1:Wdo], 0.0)

                def chunk_mms(s0, ns):
                    mms = []  # (tap, out_lo, out_hi, mov s-slice)
                    for t in range(9):
                        kx = t % 3
                        mms.append((t, 0, ns * Wd, s0, ns, kx))
                    for kx in range(3):      # g=0,ky=0 from strip s-1
                        if s0 == 0:
                            if ns > 1:
                                mms.append((9 + kx, Wd, ns * Wd, 0, ns - 1, kx))
                        else:
                            mms.append((9 + kx, 0, ns * Wd, s0 - 1, ns, kx))
                    for kx in range(3):      # g=G-1,ky=2 from strip s+1
                        if s0 + ns == Sf:
                            if ns > 1:
                                mms.append((12 + kx, 0, (ns - 1) * Wd,
                                            s0 + 1, ns - 1, kx))
                        else:
                            mms.append((12 + kx, 0, ns * Wd, s0 + 1, ns, kx))
                    return mms

                for g0 in range(0, len(chs), 3):
                    grp = chs[g0:g0 + 3]
                    pts = [psc.tile([colsB, 512], F32, tag="cps",
                                    name=f"cpsB{li}_{g0}_{gi}")
                           for gi in range(len(grp))]
                    plans = [chunk_mms(s0, ns) for (s0, ns) in grp]
                    for t in range(15):
                        for pt, plan in zip(pts, plans):
                            for (tt, o0, o1, ms, mn, kx) in plan:
                                if tt != t:
                                    continue
                                nc.tensor.matmul(
                                    pt[:, o0:o1], wkt[li][:, t, :],
                                    outA[:, ms:ms + mn, kx:kx + Wd],
                                    start=(t == plan[0][0]),
                                    stop=(t == plan[-1][0]))
                    for pt, (s0, ns) in zip(pts, grp):
                        nc.vector.scalar_tensor_tensor(
                            out=outB[:, s0:s0 + ns, 1:1 + Wd],
                            in0=pt[:, 0:ns * Wd],
                            scalar=bxt[li], in1=zeros_bf[0:colsB, 0:ns * Wd],
                            op0=ALU.add, op1=ALU.max)

                if out_nm == 'tok':
                    # regroup (g, co) partitions into token channels, straight
                    # into SBUF: one small SBUF->SBUF DMA per g
                    for g in range(G):
                        (nc.gpsimd if g % 2 else nc.sync).dma_start(
                            out=tokT[:, g::G, :],
                            in_=outB[g * Cout:(g + 1) * Cout, :, 1:1 + Wd])
                    return
                # partitions (co, g): per channel section, split into strip
                # halves across queues so each pool chain starts early and no
                # single queue carries a multi-us transfer
                Wpo, Goffo, blco, blo = geom[out_nm]
                wengs = [nc.gpsimd, nc.sync, nc.scalar]
                Shh = Sf // 2
                wh = ((0, Shh), (Shh, Sf - Shh)) if out_nm == 'm2' \
                    else ((0, Sf),)
                for co in range(Cout):
                    for hi, (s0, ns) in enumerate(wh):
                        dst = bass.AP(
                            tensor=plane[out_nm].tensor,
                            offset=co * blco + Goffo + (s0 * G + 1) * Wpo,
                            ap=[[Wpo, G], [G * Wpo, ns], [1, Wdo]])
                        wengs[(2 * co + hi) % 3].dma_start(
                            out=dst, in_=outB[co * G:(co + 1) * G,
                                             s0:s0 + ns, :])

            def pool_layer(in_nm, out_nm):
                C, H, Wd = PLANES[in_nm]
                Wp, Goff, blc, bl = geom[in_nm]
                H2, W2 = H // 2, Wd // 2
                Wp2, Goff2, blc2, bl2 = geom[out_nm]
                it, ot = plane[in_nm].tensor, plane[out_nm].tensor
                # one load/store per channel: output rows on partitions (H2<=128)
                for c in range(C):
                    t3 = work.tile([128, 2, Wd], BF16, tag="pool", bufs=3,
                                   name=f"pool_{in_nm}_{c}")
                    src = bass.AP(tensor=it,
                                  offset=c * blc + Goff + Wp + 1,
                                  ap=[[2 * Wp, H2], [Wp, 2], [1, Wd]])
                    (nc.sync if c % 2 else nc.scalar).dma_start(
                        out=t3[0:H2, :, :], in_=src)
                    m1t = work.tile([128, 2, W2], BF16, tag="plw", bufs=2)
                    nc.vector.tensor_max(m1t[0:H2], t3[0:H2, :, 0::2],
                                         t3[0:H2, :, 1::2])
                    m2t = work.tile([128, W2], BF16, tag="plh", bufs=2)
                    nc.vector.tensor_max(m2t[0:H2], m1t[0:H2, 0, :],
                                         m1t[0:H2, 1, :])
                    dst = bass.AP(tensor=ot,
                                  offset=c * blc2 + Goff2 + Wp2 + 1,
                                  ap=[[Wp2, H2], [1, W2]])
                    nc.gpsimd.dma_start(out=dst, in_=m2t[0:H2, :])

            scope_conv = nc.named_scope("conv"); scope_conv.__enter__()
            with tc.tile_pool(name="psc", bufs=6, space="PSUM") as psc:
                conv_pair(0, 'x', 'm2')
                pool_layer('m2', 'p1')
                conv_pair(2, 'p1', 'm4')
                pool_layer('m4', 'p2')
                conv_pair(4, 'p2', 'tok')
            scope_conv.__exit__(None, None, None)

            # ---------------- tokens + q/k/v ----------------
            scope_qkv = nc.named_scope("qkv"); scope_qkv.__enter__()
            # atlas load early (transfer overlaps qkv); the exp itself is
            # issued after the q/k copies so it rides the attention ramp-up
            Wt = const.tile([128, CF], BF16, tag="W")
            nc.sync.dma_start(out=Wt, in_=wa_d[:, :])
            EB = const.tile([128, CF], BF16, tag="EB")

            tokF = tokT.rearrange("c y x -> c (y x)")
            # two projection layouts: qkT = (q rows 0:64 | k rows 64:128) and
            # kqT = (k | q), so S matmuls can alternate PE row-halves and run
            # concurrently (row tiling: K=64 uses half the array)
            qkT = const.tile([128, N], BF16, tag="qkT")
            kqT = const.tile([128, N], BF16, tag="kqT")
            v_sb = const.tile([128, NCH, 65], BF16, tag="v")
            nc.vector.memset(v_sb, 1.0)

            with tc.tile_pool(name="psq", bufs=2, space="PSUM") as psq:
                for j in range(NQC):
                    ps_qk = psq.tile([128, 512], F32, tag="qkps")
                    nc.tensor.matmul(ps_qk, wqkv_t[:, 0:128],
                                     tokF[:, j * 512:(j + 1) * 512],
                                     start=True, stop=True)
                    nc.scalar.activation(out=qkT[:, j * 512:(j + 1) * 512],
                                         in_=ps_qk, func=AF.Copy)
                    ps_kq = psq.tile([128, 512], F32, tag="kqps")
                    nc.tensor.matmul(ps_kq, wqkv_t[:, 192:320],
                                     tokF[:, j * 512:(j + 1) * 512],
                                     start=True, stop=True)
                    nc.scalar.activation(out=kqT[:, j * 512:(j + 1) * 512],
                                         in_=ps_kq, func=AF.Copy)
                nc.scalar.activation(out=EB, in_=Wt, func=AF.Exp)
                for c in range(NCH):
                    ps_v = psq.tile([128, 64], F32, tag="vps")
                    nc.tensor.matmul(ps_v, tokF[:, c * 128:(c + 1) * 128],
                                     wqkv_t[:, 128:192], start=True, stop=True)
                    nc.vector.tensor_copy(v_sb[:, c, 0:64], ps_v)
            scope_qkv.__exit__(None, None, None)

            # ---------------- attention ----------------
            scope_attn = nc.named_scope("attn"); scope_attn.__enter__()
            # key-chunk-outer over pairs of query chunks: consecutive S (and
            # AV) matmuls share their stationary, amortizing LDWEIGHTS; slices
            # stream through rotating 3-bank PSUM tiles with one wide exp each
            with tc.tile_pool(name="pss", bufs=2, space="PSUM") as pss, \
                 tc.tile_pool(name="psa", bufs=2, space="PSUM") as psa:
                for jg in range(NQC // 2):
                    accs = [psa.tile([65, 512], F32, tag="acc",
                                     name=f"acc{jg}_{ji}") for ji in range(2)]
                    cur, pend, nseq = None, [], 0
                    for c in range(NCH):
                        for ji in range(2):
                            j = 2 * jg + ji
                            slot = nseq % 3
                            if slot == 0:
                                cur = pss.tile([128, 3, 512], F32, tag="s3",
                                               name=f"s3_{jg}_{nseq}")
                            if nseq % 2 == 0:   # PE rows 0-63
                                nc.tensor.matmul(
                                    cur[:, slot, :],
                                    kqT[0:64, c * 128:(c + 1) * 128],
                                    qkT[0:64, j * 512:(j + 1) * 512],
                                    start=True, stop=True)
                            else:               # PE rows 64-127, concurrent
                                nc.tensor.matmul(
                                    cur[:, slot, :],
                                    qkT[64:128, c * 128:(c + 1) * 128],
                                    kqT[64:128, j * 512:(j + 1) * 512],
                                    start=True, stop=True)
                            pend.append((slot, c, ji))
                            nseq += 1
                            if slot == 2 or nseq == 2 * NCH:
                                w = len(pend)
                                at3 = work.tile([128, 3, 512], BF16,
                                                tag="at", bufs=3)
                                nc.scalar.activation(out=at3[:, 0:w, :],
                                                     in_=cur[:, 0:w, :],
                                                     func=AF.Exp)
                                atb3 = work.tile([128, 3, 512], BF16,
                                                 tag="atb", bufs=3)
                                for i, (sl, cc, jji) in enumerate(pend):
                                    s0 = (8 * (2 * jg + jji) - 2 * cc + 63) * 64
                                    nc.vector.tensor_mul(atb3[:, i, :],
                                                         at3[:, i, :],
                                                         EB[:, s0:s0 + 512])
                                for i, (sl, cc, jji) in enumerate(pend):
                                    nc.tensor.matmul(
                                        accs[jji], v_sb[:, cc, :],
                                        atb3[:, i, :], start=(cc == 0),
                                        stop=(cc == NCH - 1))
                                pend = []
                    # epilogue: divide by the attention sums (row 64 of acc)
                    for ji in range(2):
                        j = 2 * jg + ji
                        acc = accs[ji]
                        sums = work.tile([1, 512], F32, tag="sums", bufs=2)
                        nc.vector.tensor_copy(sums, acc[64:65, :])
                        rcp_f = work.tile([1, 512], F32, tag="rcpf", bufs=2)
                        nc.vector.reciprocal_approx_fast(out=rcp_f, in_=sums)
                        bc_sb = work.tile([64, 512], F32, tag="bcs", bufs=2)
                        nc.gpsimd.partition_broadcast(bc_sb, rcp_f)
                        res = work.tile([64, 512], F32, tag="res", bufs=2)
                        nc.vector.tensor_mul(res, acc[0:64, :], bc_sb)
                        nc.sync.dma_start(out=out_d[:, j * 512:(j + 1) * 512],
                                          in_=res)
            scope_attn.__exit__(None, None, None)

    nc.finalize()
    _BUILD_CACHE['nc'] = nc
    return nc


def _prep_inputs(inputs):
    """Build the 8 per-core input maps (layout/packing only)."""
    x = np.asarray(inputs['x'], dtype=np.float32)
    qkv_w = np.asarray(inputs['qkv_w'], dtype=np.float32)
    table = np.asarray(inputs['bias_table'], dtype=np.float32)

    Wp, Goff, _, BLX = _lay(*PLANES['x'])
    xbufs = []
    for b in range(B):
        pad = np.zeros((258, Wp), np.float32)
        pad[1:257, 1:257] = x[b, 0]
        buf = np.zeros((1, BLX), np.float32)
        buf[0, Goff:Goff + 258 * Wp] = pad.reshape(-1)
        xbufs.append(buf.astype(BF16_NP))

    wks, bxs = [], []
    for i, (Cin, Cout, _, _, G) in enumerate(CONVS):
        w = np.asarray(inputs[f'conv{i + 1}_w'], dtype=np.float32)
        bias = np.asarray(inputs[f'conv{i + 1}_b'], dtype=np.float32)
        rows, cols = _wshape(i)
        ar = np.arange(G)
        taps = []
        if i == 0:
            for kx in range(3):
                Wk = np.zeros((rows, cols), np.float32)
                for ky in range(3):
                    for co in range(Cout):
                        Wk[ky * G + ar, ar * Cout + co] = w[co, 0, ky, kx]
                taps.append(Wk)
        elif i % 2 == 0:  # odd (loading) 9-tap layers 3, 5
            for ky in range(3):
                for kx in range(3):
                    Wk = np.zeros((rows, cols), np.float32)
                    for ci in range(Cin):
                        for co in range(Cout):
                            Wk[ci * (G + 2) + ky + ar, ar * Cout + co] = \
                                w[co, ci, ky, kx]
                    taps.append(Wk)
        else:  # even fused layers 2, 4, 6: rows index (g_r, ci)
            # layers 2/4 write per-channel pool planes: cols (co, g);
            # layer 6 writes tok per-g: cols (g, co)
            def col(g, co):
                return (g * Cout + co) if i == 5 else (co * G + g)
            for ky in range(3):
                for kx in range(3):
                    Wk = np.zeros((rows, cols), np.float32)
                    for ci in range(Cin):
                        for co in range(Cout):
                            gr = ar + ky - 1
                            sel = (gr >= 0) & (gr < G)
                            Wk[gr[sel] * Cin + ci,
                               col(ar[sel], co)] = w[co, ci, ky, kx]
                    taps.append(Wk)
            for kx in range(3):  # lo: (g=0, ky=0) from strip s-1 row G-1
                Wk = np.zeros((rows, cols), np.float32)
                for ci in range(Cin):
                    for co in range(Cout):
                        Wk[(G - 1) * Cin + ci, col(0, co)] = w[co, ci, 0, kx]
                taps.append(Wk)
            for kx in range(3):  # hi: (g=G-1, ky=2) from strip s+1 row 0
                Wk = np.zeros((rows, cols), np.float32)
                for ci in range(Cin):
                    for co in range(Cout):
                        Wk[ci, col(G - 1, co)] = w[co, ci, 2, kx]
                taps.append(Wk)
        wks.append(np.concatenate(taps, axis=1).astype(BF16_NP))
        if i in (1, 3):
            bxs.append(np.repeat(bias, G).astype(np.float32))  # (co, g)
        else:
            bxs.append(np.tile(bias, G).astype(np.float32))    # (g, co)

    atlases = []
    for h in range(NUM_HEADS):
        tab = table[:, h].reshape(2 * TABLE_M - 1, 2 * TABLE_M - 1)
        Ct = tab[96:96 + 127, 96:96 + 127]  # [127, 127]
        tmp = np.zeros((127, 128), np.float32)
        tmp[:, :127] = Ct
        cfbuf = np.zeros(191 + 16256 + 129, np.float32)
        cfbuf[191:191 + 16256] = tmp.reshape(-1)
        sw = np.lib.stride_tricks.sliding_window_view(cfbuf, 16256)
        p = np.arange(128)
        offs = 254 - (p % 64) - 128 * (p // 64)
        full = sw[offs]                                   # [128, 127*128]
        a2 = full.reshape(128, 127, 128)[:, :, 0:64].reshape(128, 127 * 64)
        atl = np.zeros((128, CF), np.float32)
        atl[:, :127 * 64] = a2
        atlases.append(atl.astype(BF16_NP))

    in_maps = []
    for core in range(8):
        b, h = core // 4, core % 4
        m = {"x": xbufs[b], "watlas": atlases[h]}
        for i in range(6):
            m[f"w{i}"] = wks[i]
            m[f"bx{i}"] = bxs[i]
        wq = qkv_w[h * 64:(h + 1) * 64, :].T * (float(DIM_HEAD) ** -0.5)
        wk = qkv_w[256 + h * 64:256 + (h + 1) * 64, :].T
        wv = qkv_w[512 + h * 64:512 + (h + 1) * 64, :].T
        m["wqkv"] = np.ascontiguousarray(
            np.concatenate([wq, wk, wv, wk, wq], axis=1)).astype(BF16_NP)
        in_maps.append(m)
    return in_maps


def kernel(_trace=False, **inputs):
    from concourse.bass_utils import run_bass_kernel_spmd
    nc = _build()
    in_maps = _prep_inputs(inputs)
    import os
    tdir = os.environ.get("KTRACE_DIR")
    if tdir:
        os.makedirs(tdir, exist_ok=True)
    res = run_bass_kernel_spmd(nc, in_maps, core_ids=list(range(8)),
                               trace=_trace, tmpdir=tdir)
    if _trace:
        kernel.last_exec_ns = res.exec_time_ns
        kernel.last_results = res
    # assemble: core -> (b, h): [64(d), 4096(n)]
    O = np.stack([np.stack([res.results[b * 4 + h]["out"] for h in range(4)])
                  for b in range(B)])                      # [B, H, 64, N]
    out = O.transpose(0, 3, 1, 2).reshape(B, N, NUM_HEADS * DIM_HEAD)
    out = out.reshape(B, GRID, GRID, NUM_HEADS * DIM_HEAD)
    shift = int(np.asarray(inputs['window_size'])) // 2
    out = np.roll(out, shift=(-shift, -shift), axis=(1, 2))
    return out.astype(np.float32)
